# revision 6
# baseline (speedup 1.0000x reference)
"""NestedMLP MoE-routed kernel for 8 TRN2 NeuronCores, fp8-accelerated.

Strategy:
  - Host routes tokens by expert (expert_mask), splits each expert's tokens
    across the 8 cores (data-parallel), pads per-core expert groups to a
    common capacity so all cores run one SPMD program.
  - Activations feature-major ([feature, token]) so both matmuls are natural
    lhsT.T @ rhs with contraction on partitions.
  - Precision plan (rel-err gate 2e-2; expert output-norm shares are ~.89/.10/
    .012/.0015 for e3..e0, so the small experts absorb fp8 noise):
      e3: bf16 both layers
      e2: L1 fp8 DoubleRow; L2 fp8 DoubleRow (AGGR) or bf16 (SAFE)
      e1: both layers fp8 DoubleRow
      e0: L1 bf16 (K=128 cannot DoubleRow), L2 fp8 DoubleRow
    fp8 weights are pre-scaled by 2^7 on the host (avoids e4m3 subnormals);
    the scale is undone at PSUM eviction (gelu scale=1/128, or the DVE fused
    (ps*1/128)+b2 for the output bias).
  - fp8 DoubleRow matmuls pack two K=128 subtiles per instruction
    (stationary [128,2,128], moving [128,2,cn]) -> 2x bf16 FLOP rate.
  - Each dma_start costs ~0.65us of serialized descriptor-issue time on its
    engine queue, so issues are spread across the sync/gpsimd/vector queues
    and batched into few large groups, ordered so each expert's tiles land
    just before its matmuls need them (small experts first, then e3's nested
    bf16 extents in k-complete column groups).
  - e0's small remainder chunk runs last so the kernel tail drains a 64-col
    slab instead of a 512-col one.
"""

import math
import os
import sys
import types

sys.path.insert(0, "/opt/trn_rl_repo")

import ml_dtypes
import numpy as np

P = 128
E = 4
D = 1024
H = 4096
OUT = 1024
NCORES = 8
MLP_RATIO = 4

BF16 = ml_dtypes.bfloat16
FP8 = ml_dtypes.float8_e4m3
SW = 128.0  # fp8 weight pre-scale (power of two)

# (d_in, d_hid, d_out) per expert
DIMS = [((D >> (E - 1 - e)), (D >> (E - 1 - e)) * MLP_RATIO, (OUT >> (E - 1 - e))) for e in range(E)]

AGGR = os.environ.get("K_MODE", "aggr") == "aggr"  # e2-L2 in fp8


def _round_up(v, m):
    return ((v + m - 1) // m) * m


def _tile_fmajor(a2d):
    """[F, C] -> [128, F//128, C] with row f = po*128 + pi."""
    f, c = a2d.shape
    return np.ascontiguousarray(a2d.reshape(f // P, P, c).transpose(1, 0, 2))


def _chunks(cap):
    plan, c0 = [], 0
    while c0 < cap:
        cn = min(512, cap - c0)
        plan.append((c0, cn))
        c0 += cn
    return plan


def _build_graph(caps):
    import concourse.mybir as mybir
    import concourse.tile as tile
    from concourse import bacc

    f32 = mybir.dt.float32
    bf16 = mybir.dt.bfloat16
    fp8 = mybir.dt.float8e4
    Gelu = mybir.ActivationFunctionType.Gelu
    DR = mybir.MatmulPerfMode.DoubleRow
    MUL = mybir.AluOpType.mult
    ADD = mybir.AluOpType.add

    ctot = sum(caps)
    offs = np.concatenate([[0], np.cumsum(caps)]).astype(int)
    cap_bf = caps[0] + caps[3]
    cap_f8 = caps[1] + caps[2]

    nc = bacc.Bacc(None, target_bir_lowering=False, debug=False)
    xtb_d = nc.declare_dram_parameter("xtb", [P, D // P, cap_bf], bf16, isOutput=False)
    xt8_d = nc.declare_dram_parameter("xt8", [P, 4, cap_f8], fp8, isOutput=False)
    w1b_d = nc.declare_dram_parameter("w1b", [P, D // P, H], bf16, isOutput=False)
    w2b_d = nc.declare_dram_parameter("w2b", [P, H // P, OUT], bf16, isOutput=False)
    w18_d = nc.declare_dram_parameter("w18", [P, 4, 2048], fp8, isOutput=False)
    w28_d = nc.declare_dram_parameter("w28", [P, 16, 512], fp8, isOutput=False)
    b1_d = nc.declare_dram_parameter("b1t", [P, H // P], f32, isOutput=False)
    b2_d = nc.declare_dram_parameter("b2t", [P, OUT // P], f32, isOutput=False)
    y_d = nc.declare_dram_parameter("yt", [P, OUT // P, ctot], bf16, isOutput=True)

    with tile.TileContext(nc) as tc:
        with (
            tc.tile_pool(name="wpool", bufs=1) as wpool,
            tc.tile_pool(name="xpool", bufs=1) as xpool,
            tc.tile_pool(name="hpool", bufs=1) as hpool,
            tc.tile_pool(name="ypool", bufs=3) as ypool,
            tc.tile_pool(name="pspool", bufs=8, space="PSUM") as pspool,
        ):
            # --- warmup: ramp the PE clock + preload the Gelu table ---
            wu = wpool.tile([P, P], bf16, tag="warmup")
            nc.vector.memset(wu[:], 0.0)
            wact = wpool.tile([P, P], bf16, tag="warmact")
            nc.scalar.activation(wact[:], wu[:], Gelu, bias=0.0)
            for _ in range(12):
                wps = pspool.tile([P, P], f32, tag="ps")
                nc.tensor.matmul(wps[:], wu[:], wu[:], start=True, stop=True)

            # --- SBUF tiles; DMA issues spread over sync/gpsimd/vector ---
            b1sb = wpool.tile([P, H // P], f32, tag="b1")
            b2sb = wpool.tile([P, OUT // P], f32, tag="b2")

            w1bx, w2bx = {}, {}

            def emit_group(eng, xdict, dram, dt, k0, k1, lo, hi, tag):
                t = wpool.tile([P, k1 - k0, hi - lo], dt, tag=tag, name=tag)
                eng.dma_start(t[:], dram[:, k0:k1, lo:hi])
                if xdict is not None:
                    for k in range(k0, k1):
                        xdict.setdefault(k, []).append((lo, hi, k0, t))
                return t

            def wslice(xdict, k, mc, width=P):
                for lo, hi, k0, t in xdict[k]:
                    if lo <= mc and mc + width <= hi:
                        return t[:, k - k0, mc - lo : mc - lo + width]
                raise AssertionError("weight slice not found")

            # sync queue: x-e0 then the w1 bf16 stream (then the y outputs)
            xe0 = xpool.tile([P, 1, caps[0]], bf16, tag="xe0")
            nc.sync.dma_start(xe0[:], xtb_d[:, :1, 0 : caps[0]])
            emit_group(nc.sync, w1bx, w1b_d, bf16, 1, 8, 0, 512, "w1b_b")
            emit_group(nc.sync, w1bx, w1b_d, bf16, 0, 8, 512, 2048, "w1b_c")
            emit_group(nc.sync, w1bx, w1b_d, bf16, 0, 8, 2048, 4096, "w1b_d")

            # scalar queue: tiny early loads, then the engine is all gelu
            emit_group(nc.scalar, w1bx, w1b_d, bf16, 0, 1, 0, 512, "w1b_a")
            nc.scalar.dma_start(b1sb[:], b1_d[:])
            nc.scalar.dma_start(b2sb[:], b2_d[:])

            # gpsimd queue: fp8 weights/x in need-order, x-e3, w2 bf16 stream
            w28t = emit_group(nc.gpsimd, None, w28_d, fp8, 0, 16, 0, 512, "w28")
            xt8 = xpool.tile([P, 4, cap_f8], fp8, tag="xt8")
            nc.gpsimd.dma_start(xt8[:], xt8_d[:])
            w18x = {}
            emit_group(nc.gpsimd, w18x, w18_d, fp8, 0, 2, 0, 1024, "w18_a")
            emit_group(nc.gpsimd, w18x, w18_d, fp8, 0, 2, 1024, 2048, "w18_b")
            emit_group(nc.gpsimd, w18x, w18_d, fp8, 2, 4, 0, 2048, "w18_c")
            xe3 = xpool.tile([P, 8, caps[3]], bf16, tag="xe3")
            nc.gpsimd.dma_start(xe3[:], xtb_d[:, :8, caps[0] : caps[0] + caps[3]])
            if AGGR:
                emit_group(nc.gpsimd, w2bx, w2b_d, bf16, 0, 32, 0, 512, "w2b_a")
                emit_group(nc.gpsimd, w2bx, w2b_d, bf16, 0, 32, 512, 1024, "w2b_b")
            else:
                emit_group(nc.gpsimd, w2bx, w2b_d, bf16, 0, 16, 0, 512, "w2b_e2")
                emit_group(nc.gpsimd, w2bx, w2b_d, bf16, 16, 32, 0, 512, "w2b_x0")
                emit_group(nc.gpsimd, w2bx, w2b_d, bf16, 0, 32, 512, 1024, "w2b_x1")

            h8 = hpool.tile([P, 16, 512], fp8, tag="h8")
            hbf = hpool.tile([P, 32, 512], bf16, tag="hbf")

            def w2pair(kp, mc):
                """[128, 2, 128] DoubleRow stationary slice of w28."""
                return w28t[:, 2 * kp : 2 * kp + 2, mc : mc + P]

            def w1pair(kp, mc):
                for lo, hi, k0, t in w18x[2 * kp]:
                    if lo <= mc and mc + P <= hi and 2 * kp + 2 - k0 <= t.shape[1]:
                        return t[:, 2 * kp - k0 : 2 * kp - k0 + 2, mc - lo : mc - lo + P]
                raise AssertionError("w18 pair slice not found")

            def evict_y(ps, m2, col, cn, scaled):
                yt = ypool.tile([P, cn], bf16, tag="yt")
                if scaled:
                    nc.vector.tensor_scalar(yt[:], ps[:], 1.0 / SW, b2sb[:, m2 : m2 + 1], MUL, ADD)
                else:
                    nc.vector.tensor_scalar_add(yt[:], ps[:], b2sb[:, m2 : m2 + 1])
                nc.sync.dma_start(y_d[:, m2, col : col + cn], yt[:])

            def expert0(c0, cn):
                col = offs[0] + c0
                for m in range(4):
                    ps = pspool.tile([P, cn], f32, tag="ps")
                    nc.tensor.matmul(ps[:], wslice(w1bx, 0, m * P), xe0[:, 0, c0 : c0 + cn], start=True, stop=True)
                    nc.scalar.activation(h8[:, m, :cn], ps[:], Gelu, bias=b1sb[:, m : m + 1])
                ps = pspool.tile([P, cn], f32, tag="ps")
                for kp in range(2):  # K=512
                    nc.tensor.matmul(
                        ps[:], w2pair(kp, 0), h8[:, 2 * kp : 2 * kp + 2, :cn],
                        start=(kp == 0), stop=(kp == 1), perf_mode=DR,
                    )
                evict_y(ps, 0, col, cn, scaled=True)

            # ---- expert 0 (first 512-chunk now; remainder after e3) ----
            e0_plan = _chunks(caps[0])
            for c0, cn in e0_plan[:1]:
                expert0(c0, cn)

            # ---- expert 1: fp8 DR both layers ----
            for c0, cn in _chunks(caps[1]):
                col = offs[1] + c0
                for m in range(8):
                    ps = pspool.tile([P, cn], f32, tag="ps")
                    nc.tensor.matmul(
                        ps[:], w1pair(0, m * P), xt8[:, 0:2, c0 : c0 + cn],
                        start=True, stop=True, perf_mode=DR,
                    )
                    nc.scalar.activation(h8[:, m, :cn], ps[:], Gelu, bias=b1sb[:, m : m + 1], scale=1.0 / SW)
                for m2 in range(2):
                    ps = pspool.tile([P, cn], f32, tag="ps")
                    for kp in range(4):  # K=1024
                        nc.tensor.matmul(
                            ps[:], w2pair(kp, m2 * P), h8[:, 2 * kp : 2 * kp + 2, :cn],
                            start=(kp == 0), stop=(kp == 3), perf_mode=DR,
                        )
                    evict_y(ps, m2, col, cn, scaled=True)

            # ---- expert 2: L1 fp8 DR; L2 fp8 DR (AGGR) or bf16 ----
            for c0, cn in _chunks(caps[2]):
                col = offs[2] + c0
                cc = caps[1] + c0
                for m in range(16):
                    ps = pspool.tile([P, cn], f32, tag="ps")
                    for kp in range(2):  # K=512
                        nc.tensor.matmul(
                            ps[:], w1pair(kp, m * P), xt8[:, 2 * kp : 2 * kp + 2, cc : cc + cn],
                            start=(kp == 0), stop=(kp == 1), perf_mode=DR,
                        )
                    if AGGR:
                        nc.scalar.activation(h8[:, m, :cn], ps[:], Gelu, bias=b1sb[:, m : m + 1], scale=1.0 / SW)
                    else:
                        nc.scalar.activation(hbf[:, m, :cn], ps[:], Gelu, bias=b1sb[:, m : m + 1], scale=1.0 / SW)
                for m2 in range(4):
                    ps = pspool.tile([P, cn], f32, tag="ps")
                    if AGGR:
                        for kp in range(8):  # K=2048
                            nc.tensor.matmul(
                                ps[:], w2pair(kp, m2 * P), h8[:, 2 * kp : 2 * kp + 2, :cn],
                                start=(kp == 0), stop=(kp == 7), perf_mode=DR,
                            )
                        evict_y(ps, m2, col, cn, scaled=True)
                    else:
                        for k in range(16):
                            nc.tensor.matmul(
                                ps[:], wslice(w2bx, k, m2 * P), hbf[:, k, :cn],
                                start=(k == 0), stop=(k == 15),
                            )
                        evict_y(ps, m2, col, cn, scaled=False)

            # ---- expert 3: bf16 both layers ----
            for c0, cn in _chunks(caps[3]):
                col = offs[3] + c0
                for m in range(32):
                    ps = pspool.tile([P, cn], f32, tag="ps")
                    for k in range(8):
                        nc.tensor.matmul(
                            ps[:], wslice(w1bx, k, m * P), xe3[:, k, c0 : c0 + cn],
                            start=(k == 0), stop=(k == 7),
                        )
                    nc.scalar.activation(hbf[:, m, :cn], ps[:], Gelu, bias=b1sb[:, m : m + 1])
                for m2 in range(8):
                    ps = pspool.tile([P, cn], f32, tag="ps")
                    for k in range(32):
                        nc.tensor.matmul(
                            ps[:], wslice(w2bx, k, m2 * P), hbf[:, k, :cn],
                            start=(k == 0), stop=(k == 31),
                        )
                    evict_y(ps, m2, col, cn, scaled=False)

            # ---- expert 0 remainder: tiny tail chunk ----
            for c0, cn in e0_plan[1:]:
                expert0(c0, cn)

    nc.compile()
    return nc, ctot, offs


def _ensure_ntff_hook_importable():
    try:
        import antenv.axon_hooks  # noqa: F401
        return
    except ImportError:
        pass
    holder = {"hook": None}
    m = types.ModuleType("antenv.axon_hooks")
    m.set_axon_ntff_profile_hook = lambda h: holder.__setitem__("hook", h)
    m.get_axon_ntff_profile_hook = lambda: holder["hook"]
    sys.modules["antenv.axon_hooks"] = m
    try:
        from trn_agent_boot.trn_boot import _ntff_profile_via_ctypes

        m.set_axon_ntff_profile_hook(_ntff_profile_via_ctypes("/opt/axon/libaxon_pjrt.so"))
    except Exception:
        pass


def kernel(x, expert_mask, w1, b1, w2, b2):
    _ensure_ntff_hook_importable()
    from concourse.bass_utils import run_bass_kernel_spmd

    B, N, _ = x.shape
    T = B * N
    xf = np.asarray(x, dtype=np.float32).reshape(T, D)
    mask = np.asarray(expert_mask).reshape(T).astype(np.int64)

    # --- host routing ---
    ids_by_e = [np.nonzero(mask == e)[0] for e in range(E)]
    counts = [len(i) for i in ids_by_e]
    caps = [max(64, _round_up(math.ceil(c / NCORES), 64)) for c in counts]
    core_ids = [[None] * E for _ in range(NCORES)]
    for e in range(E):
        parts = np.array_split(ids_by_e[e], NCORES)
        for c in range(NCORES):
            assert len(parts[c]) <= caps[e]
            core_ids[c][e] = parts[c]

    nc, ctot, offs = _build_graph(caps)

    # --- host weight prep ---
    w1f = np.asarray(w1, np.float32)
    w2f = np.asarray(w2, np.float32)
    w1bt = _tile_fmajor(w1f.T).astype(BF16)                             # [128, 8, 4096]
    w2bt = _tile_fmajor(w2f.T).astype(BF16)                             # [128, 32, 1024]
    w18t = _tile_fmajor((w1f[:2048, :512] * SW).T).astype(FP8)          # [128, 4, 2048]
    w28t = _tile_fmajor((w2f[:512, :2048] * SW).T).astype(FP8)          # [128, 16, 512]
    b1t = np.ascontiguousarray(np.asarray(b1, np.float32).reshape(H // P, P).T)
    b2t = np.ascontiguousarray(np.asarray(b2, np.float32).reshape(OUT // P, P).T)

    cap_bf = caps[0] + caps[3]
    cap_f8 = caps[1] + caps[2]
    in_maps = []
    for c in range(NCORES):
        xgb = np.zeros((cap_bf, D), np.float32)
        ids0, ids3 = core_ids[c][0], core_ids[c][3]
        xgb[: len(ids0)] = xf[ids0]
        xgb[caps[0] : caps[0] + len(ids3)] = xf[ids3]
        xtb = _tile_fmajor(xgb.T).astype(BF16)                          # [128, 8, cap_bf]

        xg8 = np.zeros((cap_f8, 512), np.float32)
        ids1, ids2 = core_ids[c][1], core_ids[c][2]
        xg8[: len(ids1)] = xf[ids1][:, :512]
        xg8[caps[1] : caps[1] + len(ids2)] = xf[ids2][:, :512]
        xt8 = _tile_fmajor(xg8.T).astype(FP8)                           # [128, 4, cap_f8]

        in_maps.append(
            {"xtb": xtb, "xt8": xt8, "w1b": w1bt, "w2b": w2bt,
             "w18": w18t, "w28": w28t, "b1t": b1t, "b2t": b2t}
        )

    res = run_bass_kernel_spmd(nc, in_maps, list(range(NCORES)))

    # --- host output assembly ---
    y = np.zeros((T, OUT), np.float32)
    for c in range(NCORES):
        yr = np.asarray(res.results[c]["yt"]).astype(np.float32)        # [128, 8, ctot]
        yfull = yr.transpose(1, 0, 2).reshape(OUT, ctot)
        for e in range(E):
            d_out = DIMS[e][2]
            ids = core_ids[c][e]
            if len(ids):
                y[ids, :d_out] = yfull[:d_out, offs[e] : offs[e] + len(ids)].T
    return y.reshape(B, N, OUT)


# revision 8
# speedup vs baseline: 1.1596x; 1.1596x over previous
"""NestedMLP MoE-routed kernel for 8 TRN2 NeuronCores, fp8-accelerated.

Strategy:
  - Host routes tokens by expert (expert_mask), splits each expert's tokens
    across the 8 cores (data-parallel), pads per-core expert groups to a
    common capacity so all cores run one SPMD program.
  - Activations feature-major ([feature, token]) so both matmuls are natural
    lhsT.T @ rhs with contraction on partitions.
  - Precision plan (rel-err gate 2e-2; expert output-norm shares are ~.89/.10/
    .012/.0015 for e3..e0, so the small experts absorb fp8 noise):
      e3: bf16 both layers
      e2: L1 fp8 DoubleRow; L2 fp8 DoubleRow (AGGR) or bf16 (SAFE)
      e1: both layers fp8 DoubleRow
      e0: L1 bf16 (K=128 cannot DoubleRow), L2 fp8 DoubleRow
    fp8 weights are pre-scaled by 2^7 on the host (avoids e4m3 subnormals);
    the scale is undone at PSUM eviction (gelu scale=1/128, or the DVE fused
    (ps*1/128)+b2 for the output bias).
  - fp8 DoubleRow matmuls pack two K=128 subtiles per instruction
    (stationary [128,2,128], moving [128,2,cn]) -> 2x bf16 FLOP rate.
  - Each dma_start costs ~0.65us of serialized descriptor-issue time on its
    engine queue, so issues are spread across the sync/gpsimd/vector queues
    and batched into few large groups, ordered so each expert's tiles land
    just before its matmuls need them (small experts first, then e3's nested
    bf16 extents in k-complete column groups).
  - e0's small remainder chunk runs last so the kernel tail drains a 64-col
    slab instead of a 512-col one.
"""

import math
import os
import sys
import types

sys.path.insert(0, "/opt/trn_rl_repo")

import ml_dtypes
import numpy as np

P = 128
E = 4
D = 1024
H = 4096
OUT = 1024
NCORES = 8
MLP_RATIO = 4

BF16 = ml_dtypes.bfloat16
FP8 = ml_dtypes.float8_e4m3
SW = 128.0  # fp8 weight pre-scale (power of two)

# (d_in, d_hid, d_out) per expert
DIMS = [((D >> (E - 1 - e)), (D >> (E - 1 - e)) * MLP_RATIO, (OUT >> (E - 1 - e))) for e in range(E)]

AGGR = os.environ.get("K_MODE", "aggr") == "aggr"  # e2-L2 in fp8


def _round_up(v, m):
    return ((v + m - 1) // m) * m


def _tile_fmajor(a2d):
    """[F, C] -> [128, F//128, C] with row f = po*128 + pi."""
    f, c = a2d.shape
    return np.ascontiguousarray(a2d.reshape(f // P, P, c).transpose(1, 0, 2))


def _chunks(cap):
    plan, c0 = [], 0
    while c0 < cap:
        cn = min(512, cap - c0)
        plan.append((c0, cn))
        c0 += cn
    return plan


def _build_graph(caps):
    import concourse.mybir as mybir
    import concourse.tile as tile
    from concourse import bacc

    f32 = mybir.dt.float32
    bf16 = mybir.dt.bfloat16
    fp8 = mybir.dt.float8e4
    Gelu = mybir.ActivationFunctionType.Gelu
    DR = mybir.MatmulPerfMode.DoubleRow
    MUL = mybir.AluOpType.mult
    ADD = mybir.AluOpType.add

    ctot = sum(caps)
    offs = np.concatenate([[0], np.cumsum(caps)]).astype(int)
    cap_bf = caps[0] + caps[3]
    cap_f8 = caps[1] + caps[2]

    nc = bacc.Bacc(None, target_bir_lowering=False, debug=False)
    xtb_d = nc.declare_dram_parameter("xtb", [P, D // P, cap_bf], bf16, isOutput=False)
    xt8_d = nc.declare_dram_parameter("xt8", [P, 4, cap_f8], fp8, isOutput=False)
    w1b_d = nc.declare_dram_parameter("w1b", [P, D // P, H], bf16, isOutput=False)
    w2b_d = nc.declare_dram_parameter("w2b", [P, H // P, OUT], bf16, isOutput=False)
    w18_d = nc.declare_dram_parameter("w18", [P, 4, 2048], fp8, isOutput=False)
    w28_d = nc.declare_dram_parameter("w28", [P, 16, 512], fp8, isOutput=False)
    b1_d = nc.declare_dram_parameter("b1t", [P, H // P], f32, isOutput=False)
    b2_d = nc.declare_dram_parameter("b2t", [P, OUT // P], f32, isOutput=False)
    y_d = nc.declare_dram_parameter("yt", [P, OUT // P, ctot], bf16, isOutput=True)

    with tile.TileContext(nc) as tc:
        with (
            tc.tile_pool(name="wpool", bufs=1) as wpool,
            tc.tile_pool(name="xpool", bufs=1) as xpool,
            tc.tile_pool(name="hpool", bufs=1) as hpool,
            tc.tile_pool(name="ypool", bufs=3) as ypool,
            tc.tile_pool(name="pspool", bufs=8, space="PSUM") as pspool,
        ):
            # --- warmup: ramp the PE clock + preload the Gelu table ---
            wu = wpool.tile([P, P], bf16, tag="warmup")
            nc.vector.memset(wu[:], 0.0)
            wact = wpool.tile([P, P], bf16, tag="warmact")
            nc.scalar.activation(wact[:], wu[:], Gelu, bias=0.0)
            for _ in range(10):
                wps = pspool.tile([P, P], f32, tag="ps")
                nc.tensor.matmul(wps[:], wu[:], wu[:], start=True, stop=True)

            # --- SBUF tiles; DMA issues spread over sync/gpsimd/vector ---
            b1sb = wpool.tile([P, H // P], f32, tag="b1")
            b2sb = wpool.tile([P, OUT // P], f32, tag="b2")

            w1bx, w2bx = {}, {}

            def emit_group(eng, xdict, dram, dt, k0, k1, lo, hi, tag):
                t = wpool.tile([P, k1 - k0, hi - lo], dt, tag=tag, name=tag)
                eng.dma_start(t[:], dram[:, k0:k1, lo:hi])
                if xdict is not None:
                    for k in range(k0, k1):
                        xdict.setdefault(k, []).append((lo, hi, k0, t))
                return t

            def wslice(xdict, k, mc, width=P):
                for lo, hi, k0, t in xdict[k]:
                    if lo <= mc and mc + width <= hi:
                        return t[:, k - k0, mc - lo : mc - lo + width]
                raise AssertionError("weight slice not found")

            # sync queue: x-e0, then the w1 bf16 stream in 1MB k-complete
            # column groups (consumption order), then w2 bf16, then the y
            # outputs. Ring-entry order == need order: the DMA rings are
            # bandwidth-saturated for the first ~60us, so anything issued
            # early steals bandwidth from everything needed earlier.
            xe0 = xpool.tile([P, 1, caps[0]], bf16, tag="xe0")
            nc.sync.dma_start(xe0[:], xtb_d[:, :1, 0 : caps[0]])
            emit_group(nc.sync, w1bx, w1b_d, bf16, 1, 8, 0, 512, "w1b_b")
            for j in range(1, 8):
                emit_group(nc.sync, w1bx, w1b_d, bf16, 0, 8, 512 * j, 512 * (j + 1), f"w1b_g{j}")
            if AGGR:
                for j in range(4):
                    emit_group(nc.sync, w2bx, w2b_d, bf16, 0, 32, 256 * j, 256 * (j + 1), f"w2b_{j}")
            else:
                emit_group(nc.sync, w2bx, w2b_d, bf16, 0, 16, 0, 512, "w2b_e2")
                emit_group(nc.sync, w2bx, w2b_d, bf16, 16, 32, 0, 512, "w2b_x0")
                emit_group(nc.sync, w2bx, w2b_d, bf16, 0, 32, 512, 768, "w2b_x1")
                emit_group(nc.sync, w2bx, w2b_d, bf16, 0, 32, 768, 1024, "w2b_x2")

            # scalar queue: tiny early loads, then the engine is all gelu
            emit_group(nc.scalar, w1bx, w1b_d, bf16, 0, 1, 0, 512, "w1b_a")
            nc.scalar.dma_start(b1sb[:], b1_d[:])
            nc.scalar.dma_start(b2sb[:], b2_d[:])

            # gpsimd queue: fp8 weights/x in need-order, then x-e3
            w28t = emit_group(nc.gpsimd, None, w28_d, fp8, 0, 16, 0, 512, "w28")
            xt8 = xpool.tile([P, 4, cap_f8], fp8, tag="xt8")
            nc.gpsimd.dma_start(xt8[:], xt8_d[:])
            w18x = {}
            emit_group(nc.gpsimd, w18x, w18_d, fp8, 0, 2, 0, 1024, "w18_a")
            emit_group(nc.gpsimd, w18x, w18_d, fp8, 0, 2, 1024, 2048, "w18_b")
            emit_group(nc.gpsimd, w18x, w18_d, fp8, 2, 4, 0, 2048, "w18_c")
            xe3 = xpool.tile([P, 8, caps[3]], bf16, tag="xe3")
            nc.gpsimd.dma_start(xe3[:], xtb_d[:, :8, caps[0] : caps[0] + caps[3]])

            h8 = hpool.tile([P, 16, 512], fp8, tag="h8")
            hbf = hpool.tile([P, 32, 512], bf16, tag="hbf")

            def w2pair(kp, mc):
                """[128, 2, 128] DoubleRow stationary slice of w28."""
                return w28t[:, 2 * kp : 2 * kp + 2, mc : mc + P]

            def w1pair(kp, mc):
                for lo, hi, k0, t in w18x[2 * kp]:
                    if lo <= mc and mc + P <= hi and 2 * kp + 2 - k0 <= t.shape[1]:
                        return t[:, 2 * kp - k0 : 2 * kp - k0 + 2, mc - lo : mc - lo + P]
                raise AssertionError("w18 pair slice not found")

            def evict_y(ps, m2, col, cn, scaled):
                yt = ypool.tile([P, cn], bf16, tag="yt")
                if scaled:
                    nc.vector.tensor_scalar(yt[:], ps[:], 1.0 / SW, b2sb[:, m2 : m2 + 1], MUL, ADD)
                else:
                    nc.vector.tensor_scalar_add(yt[:], ps[:], b2sb[:, m2 : m2 + 1])
                nc.sync.dma_start(y_d[:, m2, col : col + cn], yt[:])

            def expert0(c0, cn):
                col = offs[0] + c0
                for m in range(4):
                    ps = pspool.tile([P, cn], f32, tag="ps")
                    nc.tensor.matmul(ps[:], wslice(w1bx, 0, m * P), xe0[:, 0, c0 : c0 + cn], start=True, stop=True)
                    nc.scalar.activation(h8[:, m, :cn], ps[:], Gelu, bias=b1sb[:, m : m + 1])
                ps = pspool.tile([P, cn], f32, tag="ps")
                for kp in range(2):  # K=512
                    nc.tensor.matmul(
                        ps[:], w2pair(kp, 0), h8[:, 2 * kp : 2 * kp + 2, :cn],
                        start=(kp == 0), stop=(kp == 1), perf_mode=DR,
                    )
                evict_y(ps, 0, col, cn, scaled=True)

            # ---- expert 0 (first 512-chunk now; remainder after e3) ----
            e0_plan = _chunks(caps[0])
            for c0, cn in e0_plan[:1]:
                expert0(c0, cn)

            # ---- expert 1: fp8 DR both layers ----
            for c0, cn in _chunks(caps[1]):
                col = offs[1] + c0
                for m in range(8):
                    ps = pspool.tile([P, cn], f32, tag="ps")
                    nc.tensor.matmul(
                        ps[:], w1pair(0, m * P), xt8[:, 0:2, c0 : c0 + cn],
                        start=True, stop=True, perf_mode=DR,
                    )
                    nc.scalar.activation(h8[:, m, :cn], ps[:], Gelu, bias=b1sb[:, m : m + 1], scale=1.0 / SW)
                for m2 in range(2):
                    ps = pspool.tile([P, cn], f32, tag="ps")
                    for kp in range(4):  # K=1024
                        nc.tensor.matmul(
                            ps[:], w2pair(kp, m2 * P), h8[:, 2 * kp : 2 * kp + 2, :cn],
                            start=(kp == 0), stop=(kp == 3), perf_mode=DR,
                        )
                    evict_y(ps, m2, col, cn, scaled=True)

            # ---- expert 2: L1 fp8 DR; L2 fp8 DR (AGGR) or bf16 ----
            for c0, cn in _chunks(caps[2]):
                col = offs[2] + c0
                cc = caps[1] + c0
                for m in range(16):
                    ps = pspool.tile([P, cn], f32, tag="ps")
                    for kp in range(2):  # K=512
                        nc.tensor.matmul(
                            ps[:], w1pair(kp, m * P), xt8[:, 2 * kp : 2 * kp + 2, cc : cc + cn],
                            start=(kp == 0), stop=(kp == 1), perf_mode=DR,
                        )
                    if AGGR:
                        nc.scalar.activation(h8[:, m, :cn], ps[:], Gelu, bias=b1sb[:, m : m + 1], scale=1.0 / SW)
                    else:
                        nc.scalar.activation(hbf[:, m, :cn], ps[:], Gelu, bias=b1sb[:, m : m + 1], scale=1.0 / SW)
                for m2 in range(4):
                    ps = pspool.tile([P, cn], f32, tag="ps")
                    if AGGR:
                        for kp in range(8):  # K=2048
                            nc.tensor.matmul(
                                ps[:], w2pair(kp, m2 * P), h8[:, 2 * kp : 2 * kp + 2, :cn],
                                start=(kp == 0), stop=(kp == 7), perf_mode=DR,
                            )
                        evict_y(ps, m2, col, cn, scaled=True)
                    else:
                        for k in range(16):
                            nc.tensor.matmul(
                                ps[:], wslice(w2bx, k, m2 * P), hbf[:, k, :cn],
                                start=(k == 0), stop=(k == 15),
                            )
                        evict_y(ps, m2, col, cn, scaled=False)

            # ---- expert 3: bf16 both layers ----
            for c0, cn in _chunks(caps[3]):
                col = offs[3] + c0
                for m in range(32):
                    ps = pspool.tile([P, cn], f32, tag="ps")
                    for k in range(8):
                        nc.tensor.matmul(
                            ps[:], wslice(w1bx, k, m * P), xe3[:, k, c0 : c0 + cn],
                            start=(k == 0), stop=(k == 7),
                        )
                    nc.scalar.activation(hbf[:, m, :cn], ps[:], Gelu, bias=b1sb[:, m : m + 1])
                for m2 in range(8):
                    ps = pspool.tile([P, cn], f32, tag="ps")
                    for k in range(32):
                        nc.tensor.matmul(
                            ps[:], wslice(w2bx, k, m2 * P), hbf[:, k, :cn],
                            start=(k == 0), stop=(k == 31),
                        )
                    evict_y(ps, m2, col, cn, scaled=False)

            # ---- expert 0 remainder: tiny tail chunk ----
            for c0, cn in e0_plan[1:]:
                expert0(c0, cn)

    nc.compile()
    return nc, ctot, offs


def _ensure_ntff_hook_importable():
    try:
        import antenv.axon_hooks  # noqa: F401
        return
    except ImportError:
        pass
    holder = {"hook": None}
    m = types.ModuleType("antenv.axon_hooks")
    m.set_axon_ntff_profile_hook = lambda h: holder.__setitem__("hook", h)
    m.get_axon_ntff_profile_hook = lambda: holder["hook"]
    sys.modules["antenv.axon_hooks"] = m
    try:
        from trn_agent_boot.trn_boot import _ntff_profile_via_ctypes

        m.set_axon_ntff_profile_hook(_ntff_profile_via_ctypes("/opt/axon/libaxon_pjrt.so"))
    except Exception:
        pass


def kernel(x, expert_mask, w1, b1, w2, b2):
    _ensure_ntff_hook_importable()
    from concourse.bass_utils import run_bass_kernel_spmd

    B, N, _ = x.shape
    T = B * N
    xf = np.asarray(x, dtype=np.float32).reshape(T, D)
    mask = np.asarray(expert_mask).reshape(T).astype(np.int64)

    # --- host routing ---
    ids_by_e = [np.nonzero(mask == e)[0] for e in range(E)]
    counts = [len(i) for i in ids_by_e]
    caps = [max(64, _round_up(math.ceil(c / NCORES), 64)) for c in counts]
    core_ids = [[None] * E for _ in range(NCORES)]
    for e in range(E):
        parts = np.array_split(ids_by_e[e], NCORES)
        for c in range(NCORES):
            assert len(parts[c]) <= caps[e]
            core_ids[c][e] = parts[c]

    nc, ctot, offs = _build_graph(caps)

    # --- host weight prep ---
    w1f = np.asarray(w1, np.float32)
    w2f = np.asarray(w2, np.float32)
    w1bt = _tile_fmajor(w1f.T).astype(BF16)                             # [128, 8, 4096]
    w2bt = _tile_fmajor(w2f.T).astype(BF16)                             # [128, 32, 1024]
    w18t = _tile_fmajor((w1f[:2048, :512] * SW).T).astype(FP8)          # [128, 4, 2048]
    w28t = _tile_fmajor((w2f[:512, :2048] * SW).T).astype(FP8)          # [128, 16, 512]
    b1t = np.ascontiguousarray(np.asarray(b1, np.float32).reshape(H // P, P).T)
    b2t = np.ascontiguousarray(np.asarray(b2, np.float32).reshape(OUT // P, P).T)

    cap_bf = caps[0] + caps[3]
    cap_f8 = caps[1] + caps[2]
    in_maps = []
    for c in range(NCORES):
        xgb = np.zeros((cap_bf, D), np.float32)
        ids0, ids3 = core_ids[c][0], core_ids[c][3]
        xgb[: len(ids0)] = xf[ids0]
        xgb[caps[0] : caps[0] + len(ids3)] = xf[ids3]
        xtb = _tile_fmajor(xgb.T).astype(BF16)                          # [128, 8, cap_bf]

        xg8 = np.zeros((cap_f8, 512), np.float32)
        ids1, ids2 = core_ids[c][1], core_ids[c][2]
        xg8[: len(ids1)] = xf[ids1][:, :512]
        xg8[caps[1] : caps[1] + len(ids2)] = xf[ids2][:, :512]
        xt8 = _tile_fmajor(xg8.T).astype(FP8)                           # [128, 4, cap_f8]

        in_maps.append(
            {"xtb": xtb, "xt8": xt8, "w1b": w1bt, "w2b": w2bt,
             "w18": w18t, "w28": w28t, "b1t": b1t, "b2t": b2t}
        )

    res = run_bass_kernel_spmd(nc, in_maps, list(range(NCORES)))

    # --- host output assembly ---
    y = np.zeros((T, OUT), np.float32)
    for c in range(NCORES):
        yr = np.asarray(res.results[c]["yt"]).astype(np.float32)        # [128, 8, ctot]
        yfull = yr.transpose(1, 0, 2).reshape(OUT, ctot)
        for e in range(E):
            d_out = DIMS[e][2]
            ids = core_ids[c][e]
            if len(ids):
                y[ids, :d_out] = yfull[:d_out, offs[e] : offs[e] + len(ids)].T
    return y.reshape(B, N, OUT)


# revision 9
# speedup vs baseline: 1.1874x; 1.0240x over previous
"""NestedMLP MoE-routed kernel for 8 TRN2 NeuronCores, fp8-accelerated.

Strategy:
  - Host routes tokens by expert (expert_mask), splits each expert's tokens
    across the 8 cores (data-parallel), pads per-core expert groups to a
    common capacity so all cores run one SPMD program.
  - Activations feature-major ([feature, token]) so both matmuls are natural
    lhsT.T @ rhs with contraction on partitions.
  - Precision plan (rel-err gate 2e-2; expert output-norm shares are ~.89/.10/
    .012/.0015 for e3..e0, so the small experts absorb fp8 noise):
      e3: bf16 both layers
      e2: L1 fp8 DoubleRow; L2 fp8 DoubleRow (AGGR) or bf16 (SAFE)
      e1: both layers fp8 DoubleRow
      e0: L1 bf16 (K=128 cannot DoubleRow), L2 fp8 DoubleRow
    fp8 weights are pre-scaled by 2^7 on the host (avoids e4m3 subnormals);
    the scale is undone at PSUM eviction (gelu scale=1/128, or the DVE fused
    (ps*1/128)+b2 for the output bias).
  - fp8 DoubleRow matmuls pack two K=128 subtiles per instruction
    (stationary [128,2,128], moving [128,2,cn]) -> 2x bf16 FLOP rate.
  - Each dma_start costs ~0.65us of serialized descriptor-issue time on its
    engine queue, so issues are spread across the sync/gpsimd/vector queues
    and batched into few large groups, ordered so each expert's tiles land
    just before its matmuls need them (small experts first, then e3's nested
    bf16 extents in k-complete column groups).
  - e0's small remainder chunk runs last so the kernel tail drains a 64-col
    slab instead of a 512-col one.
"""

import math
import os
import sys
import types

sys.path.insert(0, "/opt/trn_rl_repo")

import ml_dtypes
import numpy as np

P = 128
E = 4
D = 1024
H = 4096
OUT = 1024
NCORES = 8
MLP_RATIO = 4

BF16 = ml_dtypes.bfloat16
FP8 = ml_dtypes.float8_e4m3
SW = 128.0  # fp8 weight pre-scale (power of two)

# (d_in, d_hid, d_out) per expert
DIMS = [((D >> (E - 1 - e)), (D >> (E - 1 - e)) * MLP_RATIO, (OUT >> (E - 1 - e))) for e in range(E)]

AGGR = os.environ.get("K_MODE", "aggr") == "aggr"  # e2-L2 in fp8


def _round_up(v, m):
    return ((v + m - 1) // m) * m


def _tile_fmajor(a2d):
    """[F, C] -> [128, F//128, C] with row f = po*128 + pi."""
    f, c = a2d.shape
    return np.ascontiguousarray(a2d.reshape(f // P, P, c).transpose(1, 0, 2))


def _chunks(cap):
    plan, c0 = [], 0
    while c0 < cap:
        cn = min(512, cap - c0)
        plan.append((c0, cn))
        c0 += cn
    return plan


def _build_graph(caps):
    import concourse.mybir as mybir
    import concourse.tile as tile
    from concourse import bacc

    f32 = mybir.dt.float32
    bf16 = mybir.dt.bfloat16
    fp8 = mybir.dt.float8e4
    Gelu = mybir.ActivationFunctionType.Gelu
    DR = mybir.MatmulPerfMode.DoubleRow
    MUL = mybir.AluOpType.mult
    ADD = mybir.AluOpType.add

    ctot = sum(caps)
    offs = np.concatenate([[0], np.cumsum(caps)]).astype(int)
    cap_bf = caps[0] + caps[3]
    cap_f8 = caps[1] + caps[2]

    nc = bacc.Bacc(None, target_bir_lowering=False, debug=False)
    xtb_d = nc.declare_dram_parameter("xtb", [P, D // P, cap_bf], bf16, isOutput=False)
    xt8_d = nc.declare_dram_parameter("xt8", [P, 4, cap_f8], fp8, isOutput=False)
    w1b_d = nc.declare_dram_parameter("w1b", [P, D // P, H], bf16, isOutput=False)
    w2b_d = nc.declare_dram_parameter("w2b", [P, H // P, OUT], bf16, isOutput=False)
    w18_d = nc.declare_dram_parameter("w18", [P, 4, 2048], fp8, isOutput=False)
    w28_d = nc.declare_dram_parameter("w28", [P, 16, 512], fp8, isOutput=False)
    b1_d = nc.declare_dram_parameter("b1t", [P, H // P], f32, isOutput=False)
    b2_d = nc.declare_dram_parameter("b2t", [P, OUT // P], f32, isOutput=False)
    y_d = nc.declare_dram_parameter("yt", [P, OUT // P, ctot], bf16, isOutput=True)

    with tile.TileContext(nc) as tc:
        with (
            tc.tile_pool(name="wpool", bufs=1) as wpool,
            tc.tile_pool(name="xpool", bufs=1) as xpool,
            tc.tile_pool(name="hpool", bufs=1) as hpool,
            tc.tile_pool(name="ypool", bufs=3) as ypool,
            tc.tile_pool(name="pspool", bufs=8, space="PSUM") as pspool,
        ):
            # --- warmup: ramp the PE clock + preload the Gelu table ---
            wu = wpool.tile([P, P], bf16, tag="warmup")
            nc.vector.memset(wu[:], 0.0)
            wact = wpool.tile([P, P], bf16, tag="warmact")
            nc.scalar.activation(wact[:], wu[:], Gelu, bias=0.0)
            for _ in range(10):
                wps = pspool.tile([P, P], f32, tag="ps")
                nc.tensor.matmul(wps[:], wu[:], wu[:], start=True, stop=True)

            # --- SBUF tiles; DMA issues spread over sync/gpsimd/vector ---
            b1sb = wpool.tile([P, H // P], f32, tag="b1")
            b2sb = wpool.tile([P, OUT // P], f32, tag="b2")

            w1bx, w2bx = {}, {}

            def emit_group(eng, xdict, dram, dt, k0, k1, lo, hi, tag):
                t = wpool.tile([P, k1 - k0, hi - lo], dt, tag=tag, name=tag)
                eng.dma_start(t[:], dram[:, k0:k1, lo:hi])
                if xdict is not None:
                    for k in range(k0, k1):
                        xdict.setdefault(k, []).append((lo, hi, k0, t))
                return t

            def wslice(xdict, k, mc, width=P):
                for lo, hi, k0, t in xdict[k]:
                    if lo <= mc and mc + width <= hi:
                        return t[:, k - k0, mc - lo : mc - lo + width]
                raise AssertionError("weight slice not found")

            # The DMA rings are bandwidth-saturated for the first ~60us and
            # transfers complete roughly in ring-entry (issue-time) order, so
            # issue EVERYTHING on one queue in exact need order: small early
            # tiles -> x-e3 -> w1 bf16 stream (1MB k-complete column groups)
            # -> w2 bf16 -> y outputs. Only the three tiny loads needed in
            # the first microseconds go on the scalar queue in parallel
            # (issue slots, not bandwidth).
            xe0 = xpool.tile([P, 1, caps[0]], bf16, tag="xe0")
            nc.sync.dma_start(xe0[:], xtb_d[:, :1, 0 : caps[0]])
            w28t = emit_group(nc.sync, None, w28_d, fp8, 0, 16, 0, 512, "w28")
            xt8 = xpool.tile([P, 4, cap_f8], fp8, tag="xt8")
            nc.sync.dma_start(xt8[:], xt8_d[:])
            w18x = {}
            emit_group(nc.sync, w18x, w18_d, fp8, 0, 2, 0, 1024, "w18_a")
            emit_group(nc.sync, w18x, w18_d, fp8, 0, 2, 1024, 2048, "w18_b")
            emit_group(nc.sync, w18x, w18_d, fp8, 2, 4, 0, 2048, "w18_c")
            xe3 = xpool.tile([P, 8, caps[3]], bf16, tag="xe3")
            nc.sync.dma_start(xe3[:], xtb_d[:, :8, caps[0] : caps[0] + caps[3]])
            emit_group(nc.sync, w1bx, w1b_d, bf16, 1, 8, 0, 512, "w1b_b")
            for j in range(1, 8):
                emit_group(nc.sync, w1bx, w1b_d, bf16, 0, 8, 512 * j, 512 * (j + 1), f"w1b_g{j}")
            if AGGR:
                for j in range(4):
                    emit_group(nc.sync, w2bx, w2b_d, bf16, 0, 32, 256 * j, 256 * (j + 1), f"w2b_{j}")
            else:
                emit_group(nc.sync, w2bx, w2b_d, bf16, 0, 16, 0, 512, "w2b_e2")
                emit_group(nc.sync, w2bx, w2b_d, bf16, 16, 32, 0, 512, "w2b_x0")
                emit_group(nc.sync, w2bx, w2b_d, bf16, 0, 32, 512, 768, "w2b_x1")
                emit_group(nc.sync, w2bx, w2b_d, bf16, 0, 32, 768, 1024, "w2b_x2")

            # scalar queue: tiny early loads, then the engine is all gelu
            emit_group(nc.scalar, w1bx, w1b_d, bf16, 0, 1, 0, 512, "w1b_a")
            nc.scalar.dma_start(b1sb[:], b1_d[:])
            nc.scalar.dma_start(b2sb[:], b2_d[:])

            h8 = hpool.tile([P, 16, 512], fp8, tag="h8")
            hbf = hpool.tile([P, 32, 512], bf16, tag="hbf")

            def w2pair(kp, mc):
                """[128, 2, 128] DoubleRow stationary slice of w28."""
                return w28t[:, 2 * kp : 2 * kp + 2, mc : mc + P]

            def w1pair(kp, mc):
                for lo, hi, k0, t in w18x[2 * kp]:
                    if lo <= mc and mc + P <= hi and 2 * kp + 2 - k0 <= t.shape[1]:
                        return t[:, 2 * kp - k0 : 2 * kp - k0 + 2, mc - lo : mc - lo + P]
                raise AssertionError("w18 pair slice not found")

            def evict_y(ps, m2, col, cn, scaled):
                yt = ypool.tile([P, cn], bf16, tag="yt")
                if scaled:
                    nc.vector.tensor_scalar(yt[:], ps[:], 1.0 / SW, b2sb[:, m2 : m2 + 1], MUL, ADD)
                else:
                    nc.vector.tensor_scalar_add(yt[:], ps[:], b2sb[:, m2 : m2 + 1])
                nc.sync.dma_start(y_d[:, m2, col : col + cn], yt[:])

            def expert0(c0, cn):
                col = offs[0] + c0
                for m in range(4):
                    ps = pspool.tile([P, cn], f32, tag="ps")
                    nc.tensor.matmul(ps[:], wslice(w1bx, 0, m * P), xe0[:, 0, c0 : c0 + cn], start=True, stop=True)
                    nc.scalar.activation(h8[:, m, :cn], ps[:], Gelu, bias=b1sb[:, m : m + 1])
                ps = pspool.tile([P, cn], f32, tag="ps")
                for kp in range(2):  # K=512
                    nc.tensor.matmul(
                        ps[:], w2pair(kp, 0), h8[:, 2 * kp : 2 * kp + 2, :cn],
                        start=(kp == 0), stop=(kp == 1), perf_mode=DR,
                    )
                evict_y(ps, 0, col, cn, scaled=True)

            # ---- expert 0 (first 512-chunk now; remainder after e3) ----
            e0_plan = _chunks(caps[0])
            for c0, cn in e0_plan[:1]:
                expert0(c0, cn)

            # ---- expert 1: fp8 DR both layers ----
            for c0, cn in _chunks(caps[1]):
                col = offs[1] + c0
                for m in range(8):
                    ps = pspool.tile([P, cn], f32, tag="ps")
                    nc.tensor.matmul(
                        ps[:], w1pair(0, m * P), xt8[:, 0:2, c0 : c0 + cn],
                        start=True, stop=True, perf_mode=DR,
                    )
                    nc.scalar.activation(h8[:, m, :cn], ps[:], Gelu, bias=b1sb[:, m : m + 1], scale=1.0 / SW)
                for m2 in range(2):
                    ps = pspool.tile([P, cn], f32, tag="ps")
                    for kp in range(4):  # K=1024
                        nc.tensor.matmul(
                            ps[:], w2pair(kp, m2 * P), h8[:, 2 * kp : 2 * kp + 2, :cn],
                            start=(kp == 0), stop=(kp == 3), perf_mode=DR,
                        )
                    evict_y(ps, m2, col, cn, scaled=True)

            # ---- expert 2: L1 fp8 DR; L2 fp8 DR (AGGR) or bf16 ----
            for c0, cn in _chunks(caps[2]):
                col = offs[2] + c0
                cc = caps[1] + c0
                for m in range(16):
                    ps = pspool.tile([P, cn], f32, tag="ps")
                    for kp in range(2):  # K=512
                        nc.tensor.matmul(
                            ps[:], w1pair(kp, m * P), xt8[:, 2 * kp : 2 * kp + 2, cc : cc + cn],
                            start=(kp == 0), stop=(kp == 1), perf_mode=DR,
                        )
                    if AGGR:
                        nc.scalar.activation(h8[:, m, :cn], ps[:], Gelu, bias=b1sb[:, m : m + 1], scale=1.0 / SW)
                    else:
                        nc.scalar.activation(hbf[:, m, :cn], ps[:], Gelu, bias=b1sb[:, m : m + 1], scale=1.0 / SW)
                for m2 in range(4):
                    ps = pspool.tile([P, cn], f32, tag="ps")
                    if AGGR:
                        for kp in range(8):  # K=2048
                            nc.tensor.matmul(
                                ps[:], w2pair(kp, m2 * P), h8[:, 2 * kp : 2 * kp + 2, :cn],
                                start=(kp == 0), stop=(kp == 7), perf_mode=DR,
                            )
                        evict_y(ps, m2, col, cn, scaled=True)
                    else:
                        for k in range(16):
                            nc.tensor.matmul(
                                ps[:], wslice(w2bx, k, m2 * P), hbf[:, k, :cn],
                                start=(k == 0), stop=(k == 15),
                            )
                        evict_y(ps, m2, col, cn, scaled=False)

            # ---- expert 3: bf16 both layers ----
            for c0, cn in _chunks(caps[3]):
                col = offs[3] + c0
                for m in range(32):
                    ps = pspool.tile([P, cn], f32, tag="ps")
                    for k in range(8):
                        nc.tensor.matmul(
                            ps[:], wslice(w1bx, k, m * P), xe3[:, k, c0 : c0 + cn],
                            start=(k == 0), stop=(k == 7),
                        )
                    nc.scalar.activation(hbf[:, m, :cn], ps[:], Gelu, bias=b1sb[:, m : m + 1])
                for m2 in range(8):
                    ps = pspool.tile([P, cn], f32, tag="ps")
                    for k in range(32):
                        nc.tensor.matmul(
                            ps[:], wslice(w2bx, k, m2 * P), hbf[:, k, :cn],
                            start=(k == 0), stop=(k == 31),
                        )
                    evict_y(ps, m2, col, cn, scaled=False)

            # ---- expert 0 remainder: tiny tail chunk ----
            for c0, cn in e0_plan[1:]:
                expert0(c0, cn)

    nc.compile()
    return nc, ctot, offs


def _ensure_ntff_hook_importable():
    try:
        import antenv.axon_hooks  # noqa: F401
        return
    except ImportError:
        pass
    holder = {"hook": None}
    m = types.ModuleType("antenv.axon_hooks")
    m.set_axon_ntff_profile_hook = lambda h: holder.__setitem__("hook", h)
    m.get_axon_ntff_profile_hook = lambda: holder["hook"]
    sys.modules["antenv.axon_hooks"] = m
    try:
        from trn_agent_boot.trn_boot import _ntff_profile_via_ctypes

        m.set_axon_ntff_profile_hook(_ntff_profile_via_ctypes("/opt/axon/libaxon_pjrt.so"))
    except Exception:
        pass


def kernel(x, expert_mask, w1, b1, w2, b2):
    _ensure_ntff_hook_importable()
    from concourse.bass_utils import run_bass_kernel_spmd

    B, N, _ = x.shape
    T = B * N
    xf = np.asarray(x, dtype=np.float32).reshape(T, D)
    mask = np.asarray(expert_mask).reshape(T).astype(np.int64)

    # --- host routing ---
    ids_by_e = [np.nonzero(mask == e)[0] for e in range(E)]
    counts = [len(i) for i in ids_by_e]
    caps = [max(64, _round_up(math.ceil(c / NCORES), 64)) for c in counts]
    core_ids = [[None] * E for _ in range(NCORES)]
    for e in range(E):
        parts = np.array_split(ids_by_e[e], NCORES)
        for c in range(NCORES):
            assert len(parts[c]) <= caps[e]
            core_ids[c][e] = parts[c]

    nc, ctot, offs = _build_graph(caps)

    # --- host weight prep ---
    w1f = np.asarray(w1, np.float32)
    w2f = np.asarray(w2, np.float32)
    w1bt = _tile_fmajor(w1f.T).astype(BF16)                             # [128, 8, 4096]
    w2bt = _tile_fmajor(w2f.T).astype(BF16)                             # [128, 32, 1024]
    w18t = _tile_fmajor((w1f[:2048, :512] * SW).T).astype(FP8)          # [128, 4, 2048]
    w28t = _tile_fmajor((w2f[:512, :2048] * SW).T).astype(FP8)          # [128, 16, 512]
    b1t = np.ascontiguousarray(np.asarray(b1, np.float32).reshape(H // P, P).T)
    b2t = np.ascontiguousarray(np.asarray(b2, np.float32).reshape(OUT // P, P).T)

    cap_bf = caps[0] + caps[3]
    cap_f8 = caps[1] + caps[2]
    in_maps = []
    for c in range(NCORES):
        xgb = np.zeros((cap_bf, D), np.float32)
        ids0, ids3 = core_ids[c][0], core_ids[c][3]
        xgb[: len(ids0)] = xf[ids0]
        xgb[caps[0] : caps[0] + len(ids3)] = xf[ids3]
        xtb = _tile_fmajor(xgb.T).astype(BF16)                          # [128, 8, cap_bf]

        xg8 = np.zeros((cap_f8, 512), np.float32)
        ids1, ids2 = core_ids[c][1], core_ids[c][2]
        xg8[: len(ids1)] = xf[ids1][:, :512]
        xg8[caps[1] : caps[1] + len(ids2)] = xf[ids2][:, :512]
        xt8 = _tile_fmajor(xg8.T).astype(FP8)                           # [128, 4, cap_f8]

        in_maps.append(
            {"xtb": xtb, "xt8": xt8, "w1b": w1bt, "w2b": w2bt,
             "w18": w18t, "w28": w28t, "b1t": b1t, "b2t": b2t}
        )

    res = run_bass_kernel_spmd(nc, in_maps, list(range(NCORES)))

    # --- host output assembly ---
    y = np.zeros((T, OUT), np.float32)
    for c in range(NCORES):
        yr = np.asarray(res.results[c]["yt"]).astype(np.float32)        # [128, 8, ctot]
        yfull = yr.transpose(1, 0, 2).reshape(OUT, ctot)
        for e in range(E):
            d_out = DIMS[e][2]
            ids = core_ids[c][e]
            if len(ids):
                y[ids, :d_out] = yfull[:d_out, offs[e] : offs[e] + len(ids)].T
    return y.reshape(B, N, OUT)
